# revision 1
# baseline (speedup 1.0000x reference)
"""Trainium2 Bass kernel for nn_CausalAttention_73212012527759.

Math (verified vs reference, see stage_sim.py):
  per position p (8192 of them): with q,k,v blocks [H=16, E=64]:
    Q = T1 @ vec(q)   (fused E-rfft 9-bin + H-DFT)      [288 = 9f*2ri*16g]
    M1 = Q*K, M2 = Q*K_ri_swapped (elementwise complex-product parts)
    corr = T2 @ [M1;M2]  (inverse H-DFT + cropped irfft16 + 1/sqrt(E))
           rows r = k*16 + a                              [256]
    E'' = exp(corr + logW[r]);  S[a] = sum_k E''[k,a]/W[a,k];  R = 1/S
    corrF = E'' * R[a]   (softmax(corr)*W, row-stochastic-ish weights)
    out[a, d] = sum_k corrF[k,a] * v[k, d]

Device mapping: feature-on-partitions, positions on the moving/free axis.
All heavy contractions = TensorE matmuls with constant fp32r stationaries.
Step5 = per-position 16x16 fp16 stationaries at 4 diagonal PE tiles, with
corrF routed through a small DRAM bounce to build the [k, (a,pos)] blocks.

Sharding: 8192 positions -> 8 cores x 1024. Host does layout only.
"""

import math
import numpy as np

import concourse.bass as bass
import concourse.bacc as bacc
import concourse.mybir as mybir
from concourse import tile
from concourse.bass_utils import run_bass_kernel_spmd

B, L, H, E = 4, 2048, 16, 64
NB = 9
NPOS = B * L
NCORES = 8
P_CORE = NPOS // NCORES      # 1024
TILE_P = 256
NT = P_CORE // TILE_P        # 4
ORDER = 0.2
SCALE = 1.0 / math.sqrt(E)
FP32 = mybir.dt.float32
FP32R = mybir.dt.float32r
FP16 = mybir.dt.float16

REPS = 1  # repeat compute in-NEFF (for timing)


def _build_constants():
    e = np.arange(E)[:, None]
    f = np.arange(NB)[None, :]
    Cc = np.exp(-2j * np.pi * e * f / E)
    g = np.arange(H)[:, None]
    h = np.arange(H)[None, :]
    Fc = np.exp(-2j * np.pi * g * h / H)
    T1 = np.zeros((H * E, 288))
    prod = np.einsum('gh,ef->hegf', Fc, Cc)
    for ff in range(NB):
        for ri in range(2):
            for gg in range(H):
                col = ff * 32 + ri * 16 + gg
                vals = prod[:, :, gg, ff]
                T1[:, col] = (vals.real if ri == 0 else vals.imag).reshape(-1)

    R18 = np.zeros((2 * NB, 16))
    for ff in range(NB):
        b_ = np.zeros(NB, complex); b_[ff] = 1.0
        R18[ff] = np.fft.irfft(b_, n=16)
        b_ = np.zeros(NB, complex); b_[ff] = 1j
        R18[NB + ff] = np.fft.irfft(b_, n=16)

    a_ = np.arange(H)[:, None]
    IRe = np.cos(2 * np.pi * a_ * np.arange(H)[None, :] / H) / H
    IIm = np.sin(2 * np.pi * a_ * np.arange(H)[None, :] / H) / H
    T2 = np.zeros((576, 256))
    for ff in range(NB):
        for gg in range(H):
            cre = np.outer(IRe[:, gg], R18[ff]) + np.outer(IIm[:, gg], R18[NB + ff])
            cim = -np.outer(IIm[:, gg], R18[ff]) + np.outer(IRe[:, gg], R18[NB + ff])
            flat_re = (SCALE * cre).T.reshape(-1)   # index k*16+a
            flat_im = (SCALE * cim).T.reshape(-1)
            T2[ff * 32 + 0 * 16 + gg] += flat_re
            T2[ff * 32 + 1 * 16 + gg] += flat_re
            T2[288 + ff * 32 + 1 * 16 + gg] += flat_im
            T2[288 + ff * 32 + 0 * 16 + gg] -= flat_im

    jj = np.arange(1, H * H, dtype=np.float64)
    w = np.concatenate([[1.0], np.cumprod(np.abs(1.0 - (ORDER + 1.0) / jj))])
    W = w.reshape(H, H)
    logW = np.log(W)
    logW_rows = logW.T.reshape(-1)          # [256] at r=k*16+a
    O1 = np.zeros((256, 16))
    for k in range(H):
        for a in range(H):
            O1[k * 16 + a, a] = 1.0 / W[a, k]
    return (T1.astype(np.float32), T2.astype(np.float32),
            logW_rows.astype(np.float32), O1.astype(np.float32),
            W.astype(np.float32))


_CONSTS = None
def get_constants():
    global _CONSTS
    if _CONSTS is None:
        _CONSTS = _build_constants()
    return _CONSTS


SHUF_SWAP16 = list(range(16, 32)) + list(range(16))


def build_nc(reps=1):
    nc = bacc.Bacc("TRN2", target_bir_lowering=False, debug=False,
                   num_devices=NCORES)

    qT = nc.declare_dram_parameter("qT", [H * E, P_CORE], FP32R, isOutput=False)
    kT = nc.declare_dram_parameter("kT", [H * E, P_CORE], FP32R, isOutput=False)
    # v packed [64 rows = q4*16+k, NT * 64p' * 64d] fp32 (cast to fp16 on load)
    vp = nc.declare_dram_parameter("vp", [64, NT * 64 * E], FP32, isOutput=False)
    t1 = nc.declare_dram_parameter("t1", [128, 8 * 3 * 96], FP32R, isOutput=False)
    t2 = nc.declare_dram_parameter("t2", [128, 6 * 256], FP32R, isOutput=False)
    lw = nc.declare_dram_parameter("lw", [128, 2], FP32, isOutput=False)
    o1 = nc.declare_dram_parameter("o1", [128, 2 * 16], FP32R, isOutput=False)
    # out: per tile: rows (q4*32 + a in 0..15), cols (batch 8, p' 8, d 64)
    out = nc.declare_dram_parameter("out", [128, NT * 8 * 512], FP32,
                                    isOutput=True)

    with tile.TileContext(nc) as tc:
        with (
            tc.tile_pool(name="const", bufs=1) as cpool,
            tc.tile_pool(name="io", bufs=2) as io,
            tc.tile_pool(name="mid", bufs=2) as mid,
            tc.tile_pool(name="dram", bufs=2, space="DRAM") as dpool,
            tc.tile_pool(name="ps_qk", bufs=1, space="PSUM") as ps_qk,
            tc.tile_pool(name="ps_c", bufs=1, space="PSUM") as ps_c,
            tc.tile_pool(name="ps_s", bufs=1, space="PSUM") as ps_s,
            tc.tile_pool(name="ps_o", bufs=2, space="PSUM") as ps_o,
        ):
            t1_sb = cpool.tile([128, 8 * 3 * 96], FP32R)
            nc.sync.dma_start(t1_sb[:], t1.ap())
            t2_sb = cpool.tile([128, 6 * 256], FP32R)
            nc.sync.dma_start(t2_sb[:], t2.ap())
            lw_sb = cpool.tile([128, 2], FP32)
            nc.sync.dma_start(lw_sb[:], lw.ap())
            o1_sb = cpool.tile([128, 2 * 16], FP32R)
            nc.sync.dma_start(o1_sb[:], o1.ap())

            qT_r = qT.ap().rearrange("(c p) (t x) -> p c x t", p=128, t=NT)
            kT_r = kT.ap().rearrange("(c p) (t x) -> p c x t", p=128, t=NT)
            vp_r = vp.ap().rearrange("(q p) (t x) -> q p x t", p=16, t=NT)
            out_r = out.ap().rearrange("p (t x) -> p x t", t=NT)

            for rep in range(reps):
              for t in range(NT):
                # ---- loads ----
                q_sb = io.tile([128, 8 * TILE_P], FP32R, tag="q")
                k_sb = io.tile([128, 8 * TILE_P], FP32R, tag="k")
                nc.sync.dma_start(q_sb[:].rearrange("p (c x) -> p c x", c=8),
                                  qT_r[:, :, :, t])
                nc.sync.dma_start(k_sb[:].rearrange("p (c x) -> p c x", c=8),
                                  kT_r[:, :, :, t])
                xv = io.tile([128, 64 * E], FP16, tag="xv")  # rows q4*32+k
                for q4 in range(4):
                    nc.gpsimd.dma_start(
                        xv[32 * q4:32 * q4 + 16, :], vp_r[q4, :, :, t])

                # ---- S24 ----
                qps = ps_qk.tile([128, 3 * TILE_P], FP32, tag="qps")
                kps = ps_qk.tile([128, 3 * TILE_P], FP32, tag="kps")
                t1_4 = t1_sb[:].rearrange("p (c a m) -> p c a m", c=8, a=3)
                for (src, dst) in ((q_sb, qps), (k_sb, kps)):
                    s3 = src[:].rearrange("p (c x) -> p c x", c=8)
                    for area in range(3):
                        for c in range(8):
                            nc.tensor.matmul(
                                dst[0:96, bass.ts(area, TILE_P)],
                                t1_4[:, c, area, :],
                                s3[:, c, :],
                                start=(c == 0), stop=(c == 7))

                # ---- products ----
                m1 = mid.tile([128, 3 * TILE_P], FP32R, tag="m1")
                m2 = mid.tile([128, 3 * TILE_P], FP32R, tag="m2")
                ksb = mid.tile([128, 3 * TILE_P], FP32, tag="ksb")
                k2 = mid.tile([128, 3 * TILE_P], FP32, tag="k2")
                nc.scalar.copy(ksb[0:96, :], kps[0:96, :])
                nc.vector.tensor_mul(m1[0:96, :], qps[0:96, :], ksb[0:96, :])
                nc.vector.stream_shuffle(k2[0:96, :], ksb[0:96, :], SHUF_SWAP16)
                nc.vector.tensor_mul(m2[0:96, :], qps[0:96, :], k2[0:96, :])

                # ---- S6: corr ----
                cps = ps_c.tile([128, 2 * TILE_P], FP32, tag="cps")
                t2_4 = t2_sb[:].rearrange("p (m h x) -> p m h x", m=6, h=2)
                for half in range(2):
                    mi = 0
                    for (msrc, base) in ((m1, 0), (m2, 3)):
                        for area in range(3):
                            nc.tensor.matmul(
                                cps[:, bass.ts(half, TILE_P)],
                                t2_4[0:96, base + area, half, :],
                                msrc[0:96, bass.ts(area, TILE_P)],
                                start=(mi == 0), stop=(mi == 5))
                            mi += 1

                # ---- softmax pieces ----
                esb = mid.tile([128, 2 * TILE_P], FP32R, tag="esb")
                for half in range(2):
                    nc.scalar.activation(
                        esb[:, bass.ts(half, TILE_P)],
                        cps[:, bass.ts(half, TILE_P)],
                        mybir.ActivationFunctionType.Exp,
                        bias=lw_sb[:, half:half + 1], scale=1.0)
                sps = ps_s.tile([128, TILE_P], FP32, tag="sps")
                o1_3 = o1_sb[:].rearrange("p (h x) -> p h x", h=2)
                for half in range(2):
                    nc.tensor.matmul(
                        sps[0:16, :], o1_3[:, half, :],
                        esb[:, bass.ts(half, TILE_P)],
                        start=(half == 0), stop=(half == 1))
                rsb = mid.tile([128, TILE_P], FP32, tag="rsb")
                nc.vector.reciprocal(rsb[0:16, :], sps[0:16, :])

                # ---- Rrep via DRAM bounce ----
                r_d = dpool.tile([16, TILE_P], FP32, tag="rd")
                nc.sync.dma_start(r_d[:], rsb[0:16, :])
                rrep = mid.tile([128, 2 * TILE_P], FP32, tag="rrep")
                from concourse.ap import AP as _AP
                _r = r_d[:]
                _rs = _AP(tensor=_r.tensor, offset=_r.offset,
                          ap=[[TILE_P, 16], [0, 2], [1, TILE_P]])
                for kk in range(8):
                    nc.sync.dma_start(
                        rrep[16 * kk:16 * kk + 16, :].rearrange(
                            "a (h x) -> a h x", h=2), _rs)

                # corrF (fp16) = E'' * Rrep
                cf = mid.tile([128, 2 * TILE_P], FP16, tag="cf")
                nc.vector.tensor_mul(cf[:], esb[:].bitcast(FP32), rrep[:])

                # ---- corrF DRAM bounce out: rows (half*128 + r) ----
                cf_d = dpool.tile([256, TILE_P], FP16, tag="cfd")
                nc.sync.dma_start(
                    cf_d[:].rearrange("(h r) x -> r h x", h=2),
                    cf[:].rearrange("r (h x) -> r h x", h=2))

                # ---- Xc read back: rows q4*32+k, free (a 16, p' 64) ----
                xc = mid.tile([128, 16 * 64], FP16, tag="xc")
                cfd_t = cf_d[:].rearrange("(k a) x -> k a x", k=16)
                for q4 in range(4):
                    nc.sync.dma_start(
                        xc[32 * q4:32 * q4 + 16, :].rearrange(
                            "k (a y) -> k a y", a=16),
                        cfd_t[:, :, bass.ts(q4, 64)])

                # ---- step5 ----
                xc_v = xc[:].rearrange("p (a y) -> p a y", a=16)
                xv_v = xv[:].rearrange("p (y d) -> p y d", y=64)
                for batch in range(8):
                    ops = ps_o.tile([128, 512], FP32, tag="ops")
                    for pp in range(8):
                        pprime = batch * 8 + pp
                        for q4 in range(4):
                            nc.tensor.matmul(
                                ops[32 * q4:32 * q4 + 16,
                                    bass.ts(pp, 64)],
                                xc_v[32 * q4:32 * q4 + 16, :, pprime],
                                xv_v[32 * q4:32 * q4 + 16, pprime, :],
                                start=True, stop=True,
                                tile_position=(32 * q4, 32 * q4))
                    ob = mid.tile([128, 512], FP32, tag="ob")
                    nc.scalar.copy(ob[:], ops[:])
                    nc.sync.dma_start(out_r[:, bass.ts(batch, 512), t], ob[:])

    nc.compile()
    return nc


_NC = {}
def get_nc(reps=1):
    if reps not in _NC:
        _NC[reps] = build_nc(reps)
    return _NC[reps]


def make_in_maps(q, k, v):
    """q,k,v: [NPOS, H, E] fp32 -> list of per-core input dicts."""
    T1c, T2c, logWc, O1c, _ = get_constants()
    t1_img = np.ascontiguousarray(
        T1c.reshape(8, 128, 288).transpose(1, 0, 2).reshape(128, 8, 3, 96)
        .reshape(128, -1))
    t2_img = np.ascontiguousarray(
        T2c.reshape(6, 96, 2, 128).transpose(1, 0, 2, 3).reshape(96, -1))
    t2_img = np.concatenate(
        [t2_img, np.zeros((32, t2_img.shape[1]), np.float32)], axis=0)
    lw_img = np.ascontiguousarray(logWc.reshape(2, 128).T)
    o1_img = np.ascontiguousarray(
        O1c.reshape(2, 128, 16).transpose(1, 0, 2).reshape(128, 32))

    in_maps = []
    for c in range(NCORES):
        sl = slice(c * P_CORE, (c + 1) * P_CORE)
        qc = q[sl].reshape(P_CORE, H * E)
        kc = k[sl].reshape(P_CORE, H * E)
        vc = v[sl]
        qTc = np.ascontiguousarray(qc.T)
        kTc = np.ascontiguousarray(kc.T)
        # vp [64 rows = q4*16+k, (t, p', d)]
        vt = vc.reshape(NT, 4, 64, H, E)              # [t, q4, p', k, d]
        vp_img = np.ascontiguousarray(
            vt.transpose(1, 3, 0, 2, 4)               # [q4, k, t, p', d]
            .reshape(64, -1))
        in_maps.append({
            "qT": qTc, "kT": kTc, "vp": vp_img,
            "t1": t1_img, "t2": t2_img, "lw": lw_img, "o1": o1_img,
        })
    return in_maps


def unpack_out(results):
    outs = []
    for c in range(NCORES):
        o = results[c]["out"].reshape(128, NT, 8, 8, 64)  # row, t, batch, p', d
        o4 = o.reshape(4, 32, NT, 8, 8, 64)[:, 0:16]      # q4, a, t, b, p', d
        # pos = t*256 + q4*64 + b*8 + p'
        oc = o4.transpose(2, 0, 3, 4, 1, 5).reshape(P_CORE, H, E)
        outs.append(oc)
    return np.concatenate(outs, axis=0).reshape(B, L, H, E)


def kernel(queries, keys, values, attn_mask=None):
    q = np.ascontiguousarray(queries, dtype=np.float32).reshape(NPOS, H, E)
    k = np.ascontiguousarray(keys, dtype=np.float32).reshape(NPOS, H, E)
    v = np.ascontiguousarray(values, dtype=np.float32).reshape(NPOS, H, E)
    in_maps = make_in_maps(q, k, v)
    nc = get_nc(REPS)
    res = run_bass_kernel_spmd(nc, in_maps, list(range(NCORES)))
    return unpack_out(res.results)


if __name__ == "__main__":
    rng = np.random.default_rng(0)
    qq = rng.standard_normal((B, L, H, E), dtype=np.float32)
    out = kernel(queries=qq, keys=qq, values=qq, attn_mask=0)
    print(out.shape, out.dtype)

